# revision 21
# baseline (speedup 1.0000x reference)
"""Multi-head attention (B=4, N=2048, C=1024, H=16, D=64) on 8 TRN2 NeuronCores.

Sharding: core c handles batch b = c//2 and head-group g = c%2 (8 heads = 512
dims). Each core computes qkv projection, attention, and a partial output
projection for its head slice; the host sums the two partials per batch and
adds the proj bias.

v4 design (v3 + flat cross-pair software pipeline):
  - all matmuls bf16; host passes x pre-transposed (xT [C, N]) and weights
    in bf16; no device-side transposes.
  - qkv projection: pairs of accumulation groups interleaved so each
    ldweights hides under the other group's matmul.
  - attention is one flat pipelined stream over (slab, pair, chunk): the
    exp'd-chunk queue carries across pair AND slab boundaries, so the PV/dn
    flush of pair p drains under pair p+1's S/exp chunks instead of as an
    idle tail (v3 lost ~3.5us per pair there).  The steady state is
    exp-engine-bound: ACT exp ~1.11us + DVE Schraudolph ~1.21us per
    [128,1024] chunk, 9/7 split.
  - output projection tiles are slotted into the pv-PSUM-bank gaps between
    pairs (slab s's proj rides inside slab s+1's stream; slab 3's at the
    end), so the exp engines never sit idle through a proj-only region
    (v3 lost ~10us per slab boundary).
  - S^T row-packed pairs (64-contraction at tile_position (0,0)/(64,0)),
    PV col-packed pairs ((0,0)/(0,64)), denominators as M=1 ones-matmul
    quads (rows {0,32}/{64,96} by chunk parity), recip + PE broadcast +
    in-place normalize off the critical path, scheduled 2 groups after the
    pair's denominator chain.
  - PSUM: st pool 3x2 banks, pv 1 (also rotates proj tiles), dn 1 = 8.
  - startup: wqkv DMA'd first (split across both queues), then xT slab 0,
    so the first qkv matmul starts ~5us earlier than slab-major order.
fp32r cannot col-tile (ISA: col_grp must be 0xf for fp32 HIGH) - bf16 is
what makes the PV/denominator packing legal.
"""

from contextlib import ExitStack

import ml_dtypes
import numpy as np

import concourse.bass as bass
import concourse.tile as tile
from concourse import bacc, mybir
from concourse.bass_utils import run_bass_kernel_spmd
from concourse.masks import make_identity

P = 128
N = 2048          # tokens per batch
C = 1024          # model dim
DC = 512          # head dims per core (8 heads x 64)
NSLABS = N // 512
F32 = mybir.dt.float32
BF16 = mybir.dt.bfloat16
I16 = mybir.dt.int16

# Schraudolph fast-exp in bf16-bit space, softmax scale 1/8 folded in:
# bf16_bits = round(logit * 0.125 * 2^7/ln2 + (127*2^7 - 486411/65536))
SCH_A = 12102203.161561485 / 65536.0 * 0.125
SCH_B = 1064866805.0 / 65536.0
DVE_CKS = frozenset(range(0, 14, 2))  # alternate ACT/DVE; last chunks on ACT (DVE does the recip tail)
DEPTH = 12  # exp'd chunks queued before PV/dn flushes chase them


def build_program(trace_label: str = "attn4"):
    nc = bacc.Bacc("TRN2", target_bir_lowering=False, name=trace_label)
    xT_d = nc.dram_tensor("xT", [C, N], BF16, kind="ExternalInput").ap()
    wqkv_d = nc.dram_tensor("wqkv", [C, 3 * DC], BF16, kind="ExternalInput").ap()
    wproj_d = nc.dram_tensor("wproj", [DC, C], BF16, kind="ExternalInput").ap()
    out_d = nc.dram_tensor("out", [N, C], F32, kind="ExternalOutput").ap()

    with tile.TileContext(nc) as tc, ExitStack() as ctx:
        _emit(ctx, tc, xT_d, wqkv_d, wproj_d, out_d)
    nc.compile()
    return nc


def _emit(ctx, tc, xT_d, wqkv_d, wproj_d, out_d):
    nc = tc.nc
    MULT = mybir.AluOpType.mult
    ADD = mybir.AluOpType.add
    BYPASS = mybir.AluOpType.bypass

    persist = ctx.enter_context(tc.tile_pool(name="persist", bufs=1))
    xT = persist.tile([P, 8, N], BF16, tag="xT")        # [c%128, c//128, n]
    wq = persist.tile([P, 8, 3 * DC], BF16, tag="wq")   # [c%128, c//128, col]
    wp = persist.tile([P, 4, C], BF16, tag="wp")        # [d%128, d//128, c]
    qT = persist.tile([P, 4, N], BF16, tag="qT")        # [d%128, pair, n]
    kT = persist.tile([P, 4, N], BF16, tag="kT")
    va = persist.tile([P, 16, DC], BF16, tag="va")      # [n%128, n//128, d]
    aT = persist.tile([P, 4, N], BF16, tag="aT")        # attn out^T

    # ---------------- consts, then DMAs, then PE warmup ----------------
    # consts first: they are tiny (~3us) but the warmup matmuls need
    # identb, and the iota/memsets run on the same engines that trigger
    # DMAs - behind 24 trigger ops they would delay the warmup ~15us.
    const = ctx.enter_context(tc.tile_pool(name="const", bufs=1))
    ident32 = const.tile([P, P], F32, tag="ident32")
    make_identity(nc, ident32)
    identb = const.tile([P, P], BF16, tag="identb")
    nc.vector.tensor_copy(identb[:], ident32[:])
    onesq = const.tile([P, 1], BF16, tag="onesq")
    nc.any.memset(onesq[:], 1.0)
    onescol = const.tile([P, 64], BF16, tag="onescol")
    nc.any.memset(onescol[:], 1.0)

    # wqkv first (the first qkv matmul needs ALL its row-chunks), then xT
    # slab by slab, then wproj; all spread over THREE trigger rings (sync/
    # scalar/gpsimd - the only DMA-capable engines, ~85 GB/s each) so the
    # per-ring drain rate doesn't serialize the input load.
    rings = [nc.sync, nc.scalar, nc.gpsimd]
    ring_i = [0]

    def dma_in(dst, src):
        rings[ring_i[0] % 3].dma_start(dst, src)
        ring_i[0] += 1

    def wq_cols(g):  # one 512-wide column group of wqkv for all row chunks
        for cc in range(8):
            dma_in(wq[:, cc, g * 512:(g + 1) * 512],
                   wqkv_d[cc * P:(cc + 1) * P, g * 512:(g + 1) * 512])

    def xt_slab(ns):
        for cc in range(8):
            dma_in(xT[:, cc, ns * 512:(ns + 1) * 512],
                   xT_d[cc * P:(cc + 1) * P, ns * 512:(ns + 1) * 512])

    wq_cols(1)          # k columns - the only weight phase 1 needs first
    xt_slab(0)
    wq_cols(2)          # v columns
    xt_slab(1)
    xt_slab(2)
    wq_cols(0)          # q columns
    xt_slab(3)
    for dc in range(4):
        dma_in(wp[:, dc, :], wproj_d[dc * P:(dc + 1) * P, :])

    with tc.tile_pool(name="ps_warm", bufs=1, space="PSUM") as ps_warm:
        warm = ps_warm.tile([P, P], F32, tag="warm")
        for _ in range(48):
            nc.tensor.matmul(warm[:], identb[:], identb[:])

    # ---------------- phase 1: qkv projection ----------------
    evac_flip = [0]

    def evac(dst, src):
        if evac_flip[0] % 2 == 0:
            nc.vector.tensor_copy(dst, src)
        else:
            nc.scalar.copy(dst, src)
        evac_flip[0] += 1

    with tc.tile_pool(name="ps1", bufs=6, space="PSUM") as ps1:
        def kq_pair(dst, colbase, dc0, ns):
            # two accumulation groups interleaved: each ldweights hides
            # under the other group's streaming matmul
            psA = ps1.tile([P, 512], F32, tag="ps1")
            psB = ps1.tile([P, 512], F32, tag="ps1")
            for cc in range(8):
                for j, ps in ((0, psA), (1, psB)):
                    col = colbase + (dc0 + j) * P
                    nc.tensor.matmul(
                        ps[:],
                        wq[:, cc, col:col + P],
                        xT[:, cc, ns * 512:(ns + 1) * 512],
                        start=(cc == 0), stop=(cc == 7),
                    )
            evac(dst[:, dc0, ns * 512:(ns + 1) * 512], psA[:])
            evac(dst[:, dc0 + 1, ns * 512:(ns + 1) * 512], psB[:])

        def v_pair(nck0):
            psA = ps1.tile([P, 512], F32, tag="ps1")
            psB = ps1.tile([P, 512], F32, tag="ps1")
            for cc in range(8):
                for j, ps in ((0, psA), (1, psB)):
                    nck = nck0 + j
                    nc.tensor.matmul(
                        ps[:],
                        xT[:, cc, nck * P:(nck + 1) * P],
                        wq[:, cc, 2 * DC:3 * DC],
                        start=(cc == 0), stop=(cc == 7),
                    )
            evac(va[:, nck0, :], psA[:])
            evac(va[:, nck0 + 1, :], psB[:])

        for ns in range(NSLABS):
            for dc0 in (0, 2):
                kq_pair(kT, DC, dc0, ns)
        for nck0 in range(0, 16, 2):
            v_pair(nck0)
        for ns in range(NSLABS):
            for dc0 in (0, 2):
                kq_pair(qT, 0, dc0, ns)

    # ---------------- phase 2: attention + proj, one flat pipeline ----------------
    with tc.tile_pool(name="st", bufs=3, space="PSUM") as st_pool, \
         tc.tile_pool(name="pv", bufs=1, space="PSUM") as pv_pool, \
         tc.tile_pool(name="dn", bufs=1, space="PSUM") as dn_pool, \
         tc.tile_pool(name="epool", bufs=18) as epool, \
         tc.tile_pool(name="nrm", bufs=2) as nrm_pool, \
         tc.tile_pool(name="oproj", bufs=2) as opool:

        queue = []        # exp'd chunks awaiting PV/dn: (s, p, ck, e)
        pair_acc = {}     # (s, p) -> (pv_tile, dn_tile), lazily allocated
        aux_sched = []    # (ready_G, thunk): chain ops spread one per boundary
        bc_sched = []     # (ready_G, s, p, aslc, rcb)
        proj_avail = []   # (s, nck, ct) proj tiles whose aT deps are emitted
        G = [0]           # global group counter

        def emit_bc(s, p, aslc, rcb, pool=None):
            # PE broadcast of the per-token reciprocal denominators (rows
            # 0/32 of rcb -> 64-row blocks), then in-place normalize of aT.
            if pool is None:
                bct = st_pool.tile([P, 2, 512], F32, tag="st",
                                   name=f"bc{s}_{p}")[:, 0, :]
            else:
                bct = pool.tile([P, 512], F32, tag="dn", name=f"bc{s}_{p}")[:]
            for sub in range(2):
                nc.tensor.matmul(
                    bct[64 * sub:64 * sub + 64, :],
                    onescol[32 * sub:32 * sub + 1, :],
                    rcb[32 * sub:32 * sub + 1, :],
                    tile_position=(32 * sub, 64 * sub),
                )
            nc.vector.scalar_tensor_tensor(
                aslc, aslc, 0.0, bct[:, :], BYPASS, MULT,
            )

        out_ring = [0]

        def emit_proj(s, nck, ct, pool):
            # one output-projection column tile, riding in a pv-/dn-bank
            # gap between that accumulator's last read and the next pair's
            # first write
            pp = pool.tile([P, 512], F32, tag=("pv" if pool is pv_pool else "dn"),
                           name=f"proj{s}_{nck}_{ct}")
            for dc in range(4):
                nc.tensor.matmul(
                    pp[:],
                    aT[:, dc, nck * P:(nck + 1) * P],
                    wp[:, dc, ct * 512:(ct + 1) * 512],
                    start=(dc == 0), stop=(dc == 3),
                )
            ot = opool.tile([P, 512], F32, tag="ot")
            evac(ot[:], pp[:])
            ring = nc.sync if out_ring[0] % 2 == 0 else nc.gpsimd
            out_ring[0] += 1
            ring.dma_start(
                out_d[nck * P:(nck + 1) * P, ct * 512:(ct + 1) * 512],
                ot[:],
            )

        def pop_proj(pool):
            if proj_avail:
                emit_proj(*proj_avail.pop(0), pool)

        def finish_pair(s, p, pv, dn):
            # The pair-end chain is spread one op per group boundary so it
            # never bunches up in front of the next chunks' exps in the
            # ACT/DVE FIFOs (each exp completion releases an st tile; a
            # bunched chain stalls S for ~1us).  pv/dn bank gaps between
            # their last read and the next pair's first write carry one
            # proj tile each.
            aslc = aT[:, p, s * 512:(s + 1) * 512]
            dsb = nrm_pool.tile([33, 512], F32, tag="dsb", name=f"dsb{s}_{p}")
            dadd = nrm_pool.tile([33, 512], F32, tag="dadd", name=f"dadd{s}_{p}")
            rc32 = nrm_pool.tile([33, 512], F32, tag="rc32", name=f"rc32_{s}_{p}")
            rcb = nrm_pool.tile([33, 512], BF16, tag="rcb", name=f"rcb{s}_{p}")

            def aux1():
                nc.scalar.copy(aslc, pv[:])
            def aux2():
                nc.scalar.copy(dsb[:], dn[64:97, :])
                nc.vector.scalar_tensor_tensor(
                    dadd[:], dn[0:33, :], 0.0, dsb[:], BYPASS, ADD,
                )
                pop_proj(pv_pool)
            def aux3():
                nc.vector.reciprocal_approx_fast(rc32[:], dadd[:])
                nc.vector.tensor_copy(rcb[:], rc32[:])
                pop_proj(dn_pool)
            aux_sched.append((G[0] + 1, aux1))
            aux_sched.append((G[0] + 2, aux2))
            aux_sched.append((G[0] + 3, aux3))
            bc_sched.append((G[0] + 5, s, p, aslc, rcb))

        def flush_batch():
            # pop up to 4 same-pair chunks; batching keeps runs of
            # same-weight-class matmuls long (each S<->PV<->dn class switch
            # exposes ~100ns of non-overlapped LDWEIGHTS)
            items = [queue.pop(0)]
            while queue and len(items) < 8 and queue[0][:2] == items[0][:2]:
                items.append(queue.pop(0))
            assert len(items) % 2 == 0
            s, p = items[0][:2]
            if (s, p) not in pair_acc:
                pair_acc[(s, p)] = (
                    pv_pool.tile([P, 512], F32, tag="pv", name=f"pv{s}_{p}"),
                    dn_pool.tile([P, 512], F32, tag="dn", name=f"dn{s}_{p}"),
                )
            pv, dn = pair_acc[(s, p)]
            for _, _, ck, e in items:
                for sub in range(2):
                    o = 64 * sub
                    h = 2 * p + sub
                    nc.tensor.matmul(
                        pv[o:o + 64, :],
                        va[:, ck, 64 * h:64 * h + 64],
                        e[:, sub, :],
                        start=(ck == 0), stop=(ck == 15),
                        tile_position=(0, o),
                    )
            for _, _, ck, e in items:
                ro = 64 * (ck % 2)
                for sub in range(2):
                    r = ro + 32 * sub
                    nc.tensor.matmul(
                        dn[r:r + 1, :],
                        onesq[:, :],
                        e[:, sub, :],
                        start=(ck < 2), stop=(ck >= 14),
                        tile_position=(0, r),
                    )
            if items[-1][2] == 15:
                finish_pair(s, p, pv, dn)

        def service():
            while aux_sched and aux_sched[0][0] <= G[0]:
                aux_sched.pop(0)[1]()
            if bc_sched and bc_sched[0][0] <= G[0]:
                _, s, p, aslc, rcb = bc_sched.pop(0)
                emit_bc(s, p, aslc, rcb)
                if p == 3 and s < 3:
                    for nck in range(4 * s, 4 * s + 4):
                        for ct in range(2):
                            proj_avail.append((s, nck, ct))

        for s in range(NSLABS):
            for p in range(4):
                for g in range(8):
                    service()
                    # flush BEFORE this group's S chunks: the flush's deps
                    # are long ready, so it executes during what would
                    # otherwise be the S chunks' st-tile wait
                    while len(queue) > DEPTH:
                        flush_batch()
                    for ck in (2 * g, 2 * g + 1):
                        # exp emitted immediately after its S chunk so the
                        # st tile is released as early as possible
                        st = st_pool.tile([P, 2, 512], F32, tag="st")
                        for sub in range(2):
                            o = 64 * sub
                            nc.tensor.matmul(
                                st[:, sub, :],
                                kT[o:o + 64, p, ck * P:(ck + 1) * P],
                                qT[o:o + 64, p, s * 512:(s + 1) * 512],
                                tile_position=(o, 0),
                            )
                        e = epool.tile([P, 2, 512], BF16, tag="e")
                        if ck in DVE_CKS or (ck == 13 and (4 * s + p) % 2 == 1):
                            nc.vector.tensor_scalar(
                                e.bitcast(I16)[:], st[:], SCH_A, SCH_B,
                                MULT, ADD,
                            )
                        else:
                            nc.scalar.activation(
                                e[:], st[:],
                                mybir.ActivationFunctionType.Exp, scale=0.125,
                            )
                        queue.append((s, p, ck, e))
                    G[0] += 1

        # ---- tail: drain the queue, remaining bc/stt, slab-3 proj ----
        while queue:
            flush_batch()
            G[0] += 1
            service()
        while aux_sched:
            aux_sched.pop(0)[1]()
        # all bc except the last pair's (its chain is still in flight on DVE)
        while len(bc_sched) > 1:
            _, s, p, aslc, rcb = bc_sched.pop(0)
            emit_bc(s, p, aslc, rcb, dn_pool)
        while proj_avail:
            emit_proj(*proj_avail.pop(0), dn_pool)
        # slab-3 proj, split accumulation: the dc=0..2 partials only need
        # pairs 0-2 normalized, so they keep the PE busy while the last
        # pair's denominator chain drains; dc=3 lands after the final stt.
        partials = []
        for nck in range(12, 15):
            pp = st_pool.tile([P, 2, 512], F32, tag="st", name=f"projt{nck}")
            for ct in range(2):
                for dc in range(3):
                    nc.tensor.matmul(
                        pp[:, ct, :],
                        aT[:, dc, nck * P:(nck + 1) * P],
                        wp[:, dc, ct * 512:(ct + 1) * 512],
                        start=(dc == 0), stop=False,
                    )
            partials.append((nck, pp))
        _, s, p, aslc, rcb = bc_sched.pop(0)
        emit_bc(s, p, aslc, rcb, dn_pool)
        for i, (nck, pp) in enumerate(partials):
            for ct in range(2):
                nc.tensor.matmul(
                    pp[:, ct, :],
                    aT[:, 3, nck * P:(nck + 1) * P],
                    wp[:, 3, ct * 512:(ct + 1) * 512],
                    start=False, stop=True,
                )
            ot = opool.tile([P, 2, 512], F32, tag="ott")
            evac(ot[:], pp[:])
            ring = nc.sync if i % 2 == 0 else nc.gpsimd
            ring.dma_start(out_d[nck * P:(nck + 1) * P, :], ot[:])
        pp = st_pool.tile([P, 2, 512], F32, tag="st", name="projt15")
        for ct in range(2):
            for dc in range(4):
                nc.tensor.matmul(
                    pp[:, ct, :],
                    aT[:, dc, 15 * P:16 * P],
                    wp[:, dc, ct * 512:(ct + 1) * 512],
                    start=(dc == 0), stop=(dc == 3),
                )
        ot = opool.tile([P, 2, 512], F32, tag="ott")
        evac(ot[:], pp[:])
        nc.gpsimd.dma_start(out_d[15 * P:16 * P, :], ot[:])


def shard_inputs(x, W_qkv, W_proj):
    """Full inputs -> 8 per-core in_maps. Core c: batch c//2, head-group c%2."""
    x = np.asarray(x, dtype=np.float32)
    W_qkv = np.asarray(W_qkv, dtype=np.float32)
    W_proj = np.asarray(W_proj, dtype=np.float32)
    bf = ml_dtypes.bfloat16
    in_maps = []
    for core in range(8):
        b, g = core // 2, core % 2
        cols = slice(g * DC, (g + 1) * DC)
        w = np.concatenate(
            [W_qkv[:, 0:C][:, cols], W_qkv[:, C:2 * C][:, cols],
             W_qkv[:, 2 * C:3 * C][:, cols]],
            axis=1,
        )
        in_maps.append({
            "xT": np.ascontiguousarray(x[b].T).astype(bf),
            "wqkv": np.ascontiguousarray(w).astype(bf),
            "wproj": np.ascontiguousarray(W_proj[g * DC:(g + 1) * DC, :]).astype(bf),
        })
    return in_maps


def unshard_output(results, b_proj):
    b_proj = np.asarray(b_proj, dtype=np.float32)
    out = np.empty((4, N, C), dtype=np.float32)
    for b in range(4):
        out[b] = results[2 * b]["out"] + results[2 * b + 1]["out"] + b_proj[None, :]
    return out


_NC_CACHE = []


def kernel(x, W_qkv, W_proj, b_proj, trace=False):
    in_maps = shard_inputs(x, W_qkv, W_proj)
    if not _NC_CACHE:
        _NC_CACHE.append(build_program())
    nc = _NC_CACHE[0]
    res = run_bass_kernel_spmd(nc, in_maps, core_ids=list(range(8)), trace=trace)
    out = unshard_output(res.results, b_proj)
    if trace:
        return out, res
    return out


# revision 22
# speedup vs baseline: 1.0046x; 1.0046x over previous
"""Multi-head attention (B=4, N=2048, C=1024, H=16, D=64) on 8 TRN2 NeuronCores.

Sharding: core c handles batch b = c//2 and head-group g = c%2 (8 heads = 512
dims). Each core computes qkv projection, attention, and a partial output
projection for its head slice; the host sums the two partials per batch and
adds the proj bias.

v4 design (v3 + flat cross-pair software pipeline):
  - all matmuls bf16; host passes x pre-transposed (xT [C, N]) and weights
    in bf16; no device-side transposes.
  - qkv projection: pairs of accumulation groups interleaved so each
    ldweights hides under the other group's matmul.
  - attention is one flat pipelined stream over (slab, pair, chunk): the
    exp'd-chunk queue carries across pair AND slab boundaries, so the PV/dn
    flush of pair p drains under pair p+1's S/exp chunks instead of as an
    idle tail (v3 lost ~3.5us per pair there).  The steady state is
    exp-engine-bound: ACT exp ~1.11us + DVE Schraudolph ~1.21us per
    [128,1024] chunk, 9/7 split.
  - output projection tiles are slotted into the pv-PSUM-bank gaps between
    pairs (slab s's proj rides inside slab s+1's stream; slab 3's at the
    end), so the exp engines never sit idle through a proj-only region
    (v3 lost ~10us per slab boundary).
  - S^T row-packed pairs (64-contraction at tile_position (0,0)/(64,0)),
    PV col-packed pairs ((0,0)/(0,64)), denominators as M=1 ones-matmul
    quads (rows {0,32}/{64,96} by chunk parity), recip + PE broadcast +
    in-place normalize off the critical path, scheduled 2 groups after the
    pair's denominator chain.
  - PSUM: st pool 3x2 banks, pv 1 (also rotates proj tiles), dn 1 = 8.
  - startup: wqkv DMA'd first (split across both queues), then xT slab 0,
    so the first qkv matmul starts ~5us earlier than slab-major order.
fp32r cannot col-tile (ISA: col_grp must be 0xf for fp32 HIGH) - bf16 is
what makes the PV/denominator packing legal.
"""

from contextlib import ExitStack

import ml_dtypes
import numpy as np

import concourse.bass as bass
import concourse.tile as tile
from concourse import bacc, mybir
from concourse.bass_utils import run_bass_kernel_spmd
from concourse.masks import make_identity

P = 128
N = 2048          # tokens per batch
C = 1024          # model dim
DC = 512          # head dims per core (8 heads x 64)
NSLABS = N // 512
F32 = mybir.dt.float32
BF16 = mybir.dt.bfloat16
I16 = mybir.dt.int16

# Schraudolph fast-exp in bf16-bit space, softmax scale 1/8 folded in:
# bf16_bits = round(logit * 0.125 * 2^7/ln2 + (127*2^7 - 486411/65536))
SCH_A = 12102203.161561485 / 65536.0 * 0.125
SCH_B = 1064866805.0 / 65536.0
DVE_CKS = frozenset(range(0, 14, 2))  # alternate ACT/DVE; last chunks on ACT (DVE does the recip tail)
DEPTH = 12  # exp'd chunks queued before PV/dn flushes chase them


def build_program(trace_label: str = "attn4"):
    nc = bacc.Bacc("TRN2", target_bir_lowering=False, name=trace_label)
    xT_d = nc.dram_tensor("xT", [C, N], BF16, kind="ExternalInput").ap()
    wqkv_d = nc.dram_tensor("wqkv", [C, 3 * DC], BF16, kind="ExternalInput").ap()
    wproj_d = nc.dram_tensor("wproj", [DC, C], BF16, kind="ExternalInput").ap()
    out_d = nc.dram_tensor("out", [N, C], F32, kind="ExternalOutput").ap()

    with tile.TileContext(nc) as tc, ExitStack() as ctx:
        _emit(ctx, tc, xT_d, wqkv_d, wproj_d, out_d)
    nc.compile()
    return nc


def _emit(ctx, tc, xT_d, wqkv_d, wproj_d, out_d):
    nc = tc.nc
    MULT = mybir.AluOpType.mult
    ADD = mybir.AluOpType.add
    BYPASS = mybir.AluOpType.bypass

    persist = ctx.enter_context(tc.tile_pool(name="persist", bufs=1))
    xT = persist.tile([P, 8, N], BF16, tag="xT")        # [c%128, c//128, n]
    wq = persist.tile([P, 8, 3 * DC], BF16, tag="wq")   # [c%128, c//128, col]
    wp = persist.tile([P, 4, C], BF16, tag="wp")        # [d%128, d//128, c]
    qT = persist.tile([P, 4, N], BF16, tag="qT")        # [d%128, pair, n]
    kT = persist.tile([P, 4, N], BF16, tag="kT")
    va = persist.tile([P, 16, DC], BF16, tag="va")      # [n%128, n//128, d]
    aT = persist.tile([P, 4, N], BF16, tag="aT")        # attn out^T

    # ---------------- consts, then DMAs, then PE warmup ----------------
    # consts first: they are tiny (~3us) but the warmup matmuls need
    # identb, and the iota/memsets run on the same engines that trigger
    # DMAs - behind 24 trigger ops they would delay the warmup ~15us.
    const = ctx.enter_context(tc.tile_pool(name="const", bufs=1))
    ident32 = const.tile([P, P], F32, tag="ident32")
    make_identity(nc, ident32)
    identb = const.tile([P, P], BF16, tag="identb")
    nc.vector.tensor_copy(identb[:], ident32[:])
    onesq = const.tile([P, 1], BF16, tag="onesq")
    nc.any.memset(onesq[:], 1.0)
    onescol = const.tile([P, 64], BF16, tag="onescol")
    nc.any.memset(onescol[:], 1.0)

    # wqkv first (the first qkv matmul needs ALL its row-chunks), then xT
    # slab by slab, then wproj; all spread over THREE trigger rings (sync/
    # scalar/gpsimd - the only DMA-capable engines, ~85 GB/s each) so the
    # per-ring drain rate doesn't serialize the input load.
    rings = [nc.sync, nc.scalar, nc.gpsimd]
    ring_i = [0]

    def dma_in(dst, src):
        rings[ring_i[0] % 3].dma_start(dst, src)
        ring_i[0] += 1

    def wq_cols(g):  # one 512-wide column group of wqkv for all row chunks
        for cc in range(8):
            dma_in(wq[:, cc, g * 512:(g + 1) * 512],
                   wqkv_d[cc * P:(cc + 1) * P, g * 512:(g + 1) * 512])

    def xt_half(h):     # half-rows: 2KB per partition line, efficient DMA
        for cc in range(8):
            dma_in(xT[:, cc, h * 1024:(h + 1) * 1024],
                   xT_d[cc * P:(cc + 1) * P, h * 1024:(h + 1) * 1024])

    wq_cols(1)          # k columns - the only weight phase 1 needs first
    xt_half(0)          # token slabs 0-1
    wq_cols(2)          # v columns
    xt_half(1)          # token slabs 2-3
    wq_cols(0)          # q columns
    for dc in range(4):
        dma_in(wp[:, dc, :], wproj_d[dc * P:(dc + 1) * P, :])

    with tc.tile_pool(name="ps_warm", bufs=1, space="PSUM") as ps_warm:
        warm = ps_warm.tile([P, P], F32, tag="warm")
        for _ in range(48):
            nc.tensor.matmul(warm[:], identb[:], identb[:])

    # ---------------- phase 1: qkv projection ----------------
    evac_flip = [0]

    def evac(dst, src):
        if evac_flip[0] % 2 == 0:
            nc.vector.tensor_copy(dst, src)
        else:
            nc.scalar.copy(dst, src)
        evac_flip[0] += 1

    with tc.tile_pool(name="ps1", bufs=6, space="PSUM") as ps1:
        def kq_pair(dst, colbase, dc0, ns):
            # two accumulation groups interleaved: each ldweights hides
            # under the other group's streaming matmul
            psA = ps1.tile([P, 512], F32, tag="ps1")
            psB = ps1.tile([P, 512], F32, tag="ps1")
            for cc in range(8):
                for j, ps in ((0, psA), (1, psB)):
                    col = colbase + (dc0 + j) * P
                    nc.tensor.matmul(
                        ps[:],
                        wq[:, cc, col:col + P],
                        xT[:, cc, ns * 512:(ns + 1) * 512],
                        start=(cc == 0), stop=(cc == 7),
                    )
            evac(dst[:, dc0, ns * 512:(ns + 1) * 512], psA[:])
            evac(dst[:, dc0 + 1, ns * 512:(ns + 1) * 512], psB[:])

        def v_pair(nck0):
            psA = ps1.tile([P, 512], F32, tag="ps1")
            psB = ps1.tile([P, 512], F32, tag="ps1")
            for cc in range(8):
                for j, ps in ((0, psA), (1, psB)):
                    nck = nck0 + j
                    nc.tensor.matmul(
                        ps[:],
                        xT[:, cc, nck * P:(nck + 1) * P],
                        wq[:, cc, 2 * DC:3 * DC],
                        start=(cc == 0), stop=(cc == 7),
                    )
            evac(va[:, nck0, :], psA[:])
            evac(va[:, nck0 + 1, :], psB[:])

        for ns in range(NSLABS):
            for dc0 in (0, 2):
                kq_pair(kT, DC, dc0, ns)
        for nck0 in range(0, 16, 2):
            v_pair(nck0)
        for ns in range(NSLABS):
            for dc0 in (0, 2):
                kq_pair(qT, 0, dc0, ns)

    # ---------------- phase 2: attention + proj, one flat pipeline ----------------
    with tc.tile_pool(name="st", bufs=3, space="PSUM") as st_pool, \
         tc.tile_pool(name="pv", bufs=1, space="PSUM") as pv_pool, \
         tc.tile_pool(name="dn", bufs=1, space="PSUM") as dn_pool, \
         tc.tile_pool(name="epool", bufs=18) as epool, \
         tc.tile_pool(name="nrm", bufs=2) as nrm_pool, \
         tc.tile_pool(name="oproj", bufs=2) as opool:

        queue = []        # exp'd chunks awaiting PV/dn: (s, p, ck, e)
        pair_acc = {}     # (s, p) -> (pv_tile, dn_tile), lazily allocated
        aux_sched = []    # (ready_G, thunk): chain ops spread one per boundary
        bc_sched = []     # (ready_G, s, p, aslc, rcb)
        proj_avail = []   # (s, nck, ct) proj tiles whose aT deps are emitted
        G = [0]           # global group counter

        def emit_bc(s, p, aslc, rcb, pool=None):
            # PE broadcast of the per-token reciprocal denominators (rows
            # 0/32 of rcb -> 64-row blocks), then in-place normalize of aT.
            if pool is None:
                bct = st_pool.tile([P, 2, 512], F32, tag="st",
                                   name=f"bc{s}_{p}")[:, 0, :]
            else:
                bct = pool.tile([P, 512], F32, tag="dn", name=f"bc{s}_{p}")[:]
            for sub in range(2):
                nc.tensor.matmul(
                    bct[64 * sub:64 * sub + 64, :],
                    onescol[32 * sub:32 * sub + 1, :],
                    rcb[32 * sub:32 * sub + 1, :],
                    tile_position=(32 * sub, 64 * sub),
                )
            nc.vector.scalar_tensor_tensor(
                aslc, aslc, 0.0, bct[:, :], BYPASS, MULT,
            )

        out_ring = [0]

        def emit_proj(s, nck, ct, pool):
            # one output-projection column tile, riding in a pv-/dn-bank
            # gap between that accumulator's last read and the next pair's
            # first write
            pp = pool.tile([P, 512], F32, tag=("pv" if pool is pv_pool else "dn"),
                           name=f"proj{s}_{nck}_{ct}")
            for dc in range(4):
                nc.tensor.matmul(
                    pp[:],
                    aT[:, dc, nck * P:(nck + 1) * P],
                    wp[:, dc, ct * 512:(ct + 1) * 512],
                    start=(dc == 0), stop=(dc == 3),
                )
            ot = opool.tile([P, 512], F32, tag="ot")
            evac(ot[:], pp[:])
            ring = nc.sync if out_ring[0] % 2 == 0 else nc.gpsimd
            out_ring[0] += 1
            ring.dma_start(
                out_d[nck * P:(nck + 1) * P, ct * 512:(ct + 1) * 512],
                ot[:],
            )

        def pop_proj(pool):
            if proj_avail:
                emit_proj(*proj_avail.pop(0), pool)

        def finish_pair(s, p, pv, dn):
            # The pair-end chain is spread one op per group boundary so it
            # never bunches up in front of the next chunks' exps in the
            # ACT/DVE FIFOs (each exp completion releases an st tile; a
            # bunched chain stalls S for ~1us).  pv/dn bank gaps between
            # their last read and the next pair's first write carry one
            # proj tile each.
            aslc = aT[:, p, s * 512:(s + 1) * 512]
            dsb = nrm_pool.tile([33, 512], F32, tag="dsb", name=f"dsb{s}_{p}")
            dadd = nrm_pool.tile([33, 512], F32, tag="dadd", name=f"dadd{s}_{p}")
            rc32 = nrm_pool.tile([33, 512], F32, tag="rc32", name=f"rc32_{s}_{p}")
            rcb = nrm_pool.tile([33, 512], BF16, tag="rcb", name=f"rcb{s}_{p}")

            def aux1():
                nc.scalar.copy(aslc, pv[:])
            def aux2():
                nc.scalar.copy(dsb[:], dn[64:97, :])
                nc.vector.scalar_tensor_tensor(
                    dadd[:], dn[0:33, :], 0.0, dsb[:], BYPASS, ADD,
                )
                pop_proj(pv_pool)
            def aux3():
                nc.vector.reciprocal_approx_fast(rc32[:], dadd[:])
                nc.vector.tensor_copy(rcb[:], rc32[:])
                pop_proj(dn_pool)
            aux_sched.append((G[0] + 1, aux1))
            aux_sched.append((G[0] + 2, aux2))
            aux_sched.append((G[0] + 3, aux3))
            bc_sched.append((G[0] + 5, s, p, aslc, rcb))

        def flush_batch():
            # pop up to 4 same-pair chunks; batching keeps runs of
            # same-weight-class matmuls long (each S<->PV<->dn class switch
            # exposes ~100ns of non-overlapped LDWEIGHTS)
            items = [queue.pop(0)]
            while queue and len(items) < 8 and queue[0][:2] == items[0][:2]:
                items.append(queue.pop(0))
            assert len(items) % 2 == 0
            s, p = items[0][:2]
            if (s, p) not in pair_acc:
                pair_acc[(s, p)] = (
                    pv_pool.tile([P, 512], F32, tag="pv", name=f"pv{s}_{p}"),
                    dn_pool.tile([P, 512], F32, tag="dn", name=f"dn{s}_{p}"),
                )
            pv, dn = pair_acc[(s, p)]
            for _, _, ck, e in items:
                for sub in range(2):
                    o = 64 * sub
                    h = 2 * p + sub
                    nc.tensor.matmul(
                        pv[o:o + 64, :],
                        va[:, ck, 64 * h:64 * h + 64],
                        e[:, sub, :],
                        start=(ck == 0), stop=(ck == 15),
                        tile_position=(0, o),
                    )
            for _, _, ck, e in items:
                ro = 64 * (ck % 2)
                for sub in range(2):
                    r = ro + 32 * sub
                    nc.tensor.matmul(
                        dn[r:r + 1, :],
                        onesq[:, :],
                        e[:, sub, :],
                        start=(ck < 2), stop=(ck >= 14),
                        tile_position=(0, r),
                    )
            if items[-1][2] == 15:
                finish_pair(s, p, pv, dn)

        def service():
            while aux_sched and aux_sched[0][0] <= G[0]:
                aux_sched.pop(0)[1]()
            if bc_sched and bc_sched[0][0] <= G[0]:
                _, s, p, aslc, rcb = bc_sched.pop(0)
                emit_bc(s, p, aslc, rcb)
                if p == 3 and s < 3:
                    for nck in range(4 * s, 4 * s + 4):
                        for ct in range(2):
                            proj_avail.append((s, nck, ct))

        for s in range(NSLABS):
            for p in range(4):
                for g in range(8):
                    service()
                    # flush BEFORE this group's S chunks: the flush's deps
                    # are long ready, so it executes during what would
                    # otherwise be the S chunks' st-tile wait
                    while len(queue) > DEPTH:
                        flush_batch()
                    for ck in (2 * g, 2 * g + 1):
                        # exp emitted immediately after its S chunk so the
                        # st tile is released as early as possible
                        st = st_pool.tile([P, 2, 512], F32, tag="st")
                        for sub in range(2):
                            o = 64 * sub
                            nc.tensor.matmul(
                                st[:, sub, :],
                                kT[o:o + 64, p, ck * P:(ck + 1) * P],
                                qT[o:o + 64, p, s * 512:(s + 1) * 512],
                                tile_position=(o, 0),
                            )
                        e = epool.tile([P, 2, 512], BF16, tag="e")
                        if ck in DVE_CKS or (ck == 13 and (4 * s + p) % 2 == 1):
                            nc.vector.tensor_scalar(
                                e.bitcast(I16)[:], st[:], SCH_A, SCH_B,
                                MULT, ADD,
                            )
                        else:
                            nc.scalar.activation(
                                e[:], st[:],
                                mybir.ActivationFunctionType.Exp, scale=0.125,
                            )
                        queue.append((s, p, ck, e))
                    G[0] += 1

        # ---- tail: drain the queue, remaining bc/stt, slab-3 proj ----
        while queue:
            flush_batch()
            G[0] += 1
            service()
        while aux_sched:
            aux_sched.pop(0)[1]()
        # all bc except the last pair's (its chain is still in flight on DVE)
        while len(bc_sched) > 1:
            _, s, p, aslc, rcb = bc_sched.pop(0)
            emit_bc(s, p, aslc, rcb, dn_pool)
        while proj_avail:
            emit_proj(*proj_avail.pop(0), dn_pool)
        # slab-3 proj, split accumulation: the dc=0..2 partials only need
        # pairs 0-2 normalized, so they keep the PE busy while the last
        # pair's denominator chain drains; dc=3 lands after the final stt.
        partials = []
        for nck in range(12, 15):
            pp = st_pool.tile([P, 2, 512], F32, tag="st", name=f"projt{nck}")
            for ct in range(2):
                for dc in range(3):
                    nc.tensor.matmul(
                        pp[:, ct, :],
                        aT[:, dc, nck * P:(nck + 1) * P],
                        wp[:, dc, ct * 512:(ct + 1) * 512],
                        start=(dc == 0), stop=False,
                    )
            partials.append((nck, pp))
        _, s, p, aslc, rcb = bc_sched.pop(0)
        emit_bc(s, p, aslc, rcb, dn_pool)
        for i, (nck, pp) in enumerate(partials):
            for ct in range(2):
                nc.tensor.matmul(
                    pp[:, ct, :],
                    aT[:, 3, nck * P:(nck + 1) * P],
                    wp[:, 3, ct * 512:(ct + 1) * 512],
                    start=False, stop=True,
                )
            ot = opool.tile([P, 2, 512], F32, tag="ott")
            evac(ot[:], pp[:])
            ring = nc.sync if i % 2 == 0 else nc.gpsimd
            ring.dma_start(out_d[nck * P:(nck + 1) * P, :], ot[:])
        pp = st_pool.tile([P, 2, 512], F32, tag="st", name="projt15")
        for ct in range(2):
            for dc in range(4):
                nc.tensor.matmul(
                    pp[:, ct, :],
                    aT[:, dc, 15 * P:16 * P],
                    wp[:, dc, ct * 512:(ct + 1) * 512],
                    start=(dc == 0), stop=(dc == 3),
                )
        ot = opool.tile([P, 2, 512], F32, tag="ott")
        evac(ot[:], pp[:])
        nc.gpsimd.dma_start(out_d[15 * P:16 * P, :], ot[:])


def shard_inputs(x, W_qkv, W_proj):
    """Full inputs -> 8 per-core in_maps. Core c: batch c//2, head-group c%2."""
    x = np.asarray(x, dtype=np.float32)
    W_qkv = np.asarray(W_qkv, dtype=np.float32)
    W_proj = np.asarray(W_proj, dtype=np.float32)
    bf = ml_dtypes.bfloat16
    in_maps = []
    for core in range(8):
        b, g = core // 2, core % 2
        cols = slice(g * DC, (g + 1) * DC)
        w = np.concatenate(
            [W_qkv[:, 0:C][:, cols], W_qkv[:, C:2 * C][:, cols],
             W_qkv[:, 2 * C:3 * C][:, cols]],
            axis=1,
        )
        in_maps.append({
            "xT": np.ascontiguousarray(x[b].T).astype(bf),
            "wqkv": np.ascontiguousarray(w).astype(bf),
            "wproj": np.ascontiguousarray(W_proj[g * DC:(g + 1) * DC, :]).astype(bf),
        })
    return in_maps


def unshard_output(results, b_proj):
    b_proj = np.asarray(b_proj, dtype=np.float32)
    out = np.empty((4, N, C), dtype=np.float32)
    for b in range(4):
        out[b] = results[2 * b]["out"] + results[2 * b + 1]["out"] + b_proj[None, :]
    return out


_NC_CACHE = []


def kernel(x, W_qkv, W_proj, b_proj, trace=False):
    in_maps = shard_inputs(x, W_qkv, W_proj)
    if not _NC_CACHE:
        _NC_CACHE.append(build_program())
    nc = _NC_CACHE[0]
    res = run_bass_kernel_spmd(nc, in_maps, core_ids=list(range(8)), trace=trace)
    out = unshard_output(res.results, b_proj)
    if trace:
        return out, res
    return out


# revision 23
# speedup vs baseline: 1.0121x; 1.0074x over previous
"""Multi-head attention (B=4, N=2048, C=1024, H=16, D=64) on 8 TRN2 NeuronCores.

Sharding: core c handles batch b = c//2 and head-group g = c%2 (8 heads = 512
dims). Each core computes qkv projection, attention, and a partial output
projection for its head slice; the host sums the two partials per batch and
adds the proj bias.

v10 design (v3 + flat cross-pair software pipeline; ~370us vs v3 ~405us,
measured unthrottled - the part runs ~20% slower when the chip enters the
P0 power state after sustained load):
  - all matmuls bf16; host passes x pre-transposed (xT [C, N]) and weights
    in bf16; no device-side transposes.
  - qkv projection: pairs of accumulation groups interleaved so each
    ldweights hides under the other group's matmul.
  - attention is one flat pipelined stream over (slab, pair, chunk): the
    exp'd-chunk queue carries across pair AND slab boundaries, so the PV/dn
    flush of pair p drains under pair p+1's S/exp chunks instead of as an
    idle tail (v3 lost ~3.5us per pair there).  PV/dn flushes pop in
    batches of up to 8 chunks and are emitted BEFORE each group's S
    chunks: long same-weight-class runs hide LDWEIGHTS (each S<->PV<->dn
    class switch exposes ~100ns), and the flush executes during what
    would otherwise be the S chunks' st-tile wait.
  - exp split ACT/DVE ~9/7 by chunk parity (8/8 on odd pairs); the
    pair-end denominator chain (pv evac, dsb+dadd, recip+cast) is spread
    one op per group boundary so it never bunches in front of the next
    chunks' exps (each exp completion releases an st tile for S).
  - output projection tiles ride in the pv-/dn-bank gaps between a pair's
    last read and the next pair's first write (slab s's proj inside slab
    s+1's stream); slab 3's proj at the tail uses split accumulation
    (dc 0-2 partials overlap the final denominator chain, dc=3 lands
    after the last stt).
  - S^T row-packed pairs (64-contraction at tile_position (0,0)/(64,0)),
    PV col-packed pairs ((0,0)/(0,64)), denominators as M=1 ones-matmul
    quads (rows {0,32}/{64,96} by chunk parity), recip + PE broadcast +
    in-place normalize off the critical path.
  - PSUM: st pool 3x2 banks, pv 1, dn 1 = 8 (proj/bc tiles rotate through
    the pv/dn/st pools in their idle windows).
  - startup: consts first (the warmup needs identb and the iota/memsets
    share engines with DMA triggers), then inputs over all three DMA
    trigger rings (sync/scalar/gpsimd, ~85 GB/s each) ordered so each
    phase's data lands just in time: wqkv k-cols, xT first half (2KB
    lines), v-cols, xT second half, q-cols, wproj.
fp32r cannot col-tile (ISA: col_grp must be 0xf for fp32 HIGH) - bf16 is
what makes the PV/denominator packing legal.  TRN2 matmul output must be
fp32 (16-bit PSUM is TRN3-only), which caps DVE exp reads at 1x mode.
"""

from contextlib import ExitStack

import ml_dtypes
import numpy as np

import concourse.bass as bass
import concourse.tile as tile
from concourse import bacc, mybir
from concourse.bass_utils import run_bass_kernel_spmd
from concourse.masks import make_identity

P = 128
N = 2048          # tokens per batch
C = 1024          # model dim
DC = 512          # head dims per core (8 heads x 64)
NSLABS = N // 512
F32 = mybir.dt.float32
BF16 = mybir.dt.bfloat16
I16 = mybir.dt.int16

# Schraudolph fast-exp in bf16-bit space, softmax scale 1/8 folded in:
# bf16_bits = round(logit * 0.125 * 2^7/ln2 + (127*2^7 - 486411/65536))
SCH_A = 12102203.161561485 / 65536.0 * 0.125
SCH_B = 1064866805.0 / 65536.0
DVE_CKS = frozenset(range(0, 14, 2))  # alternate ACT/DVE; last chunks on ACT (DVE does the recip tail)
DEPTH = 12  # exp'd chunks queued before PV/dn flushes chase them


def build_program(trace_label: str = "attn10"):
    nc = bacc.Bacc("TRN2", target_bir_lowering=False, name=trace_label)
    xT_d = nc.dram_tensor("xT", [C, N], BF16, kind="ExternalInput").ap()
    wqkv_d = nc.dram_tensor("wqkv", [C, 3 * DC], BF16, kind="ExternalInput").ap()
    wproj_d = nc.dram_tensor("wproj", [DC, C], BF16, kind="ExternalInput").ap()
    out_d = nc.dram_tensor("out", [N, C], F32, kind="ExternalOutput").ap()

    with tile.TileContext(nc) as tc, ExitStack() as ctx:
        _emit(ctx, tc, xT_d, wqkv_d, wproj_d, out_d)
    nc.compile()
    return nc


def _emit(ctx, tc, xT_d, wqkv_d, wproj_d, out_d):
    nc = tc.nc
    MULT = mybir.AluOpType.mult
    ADD = mybir.AluOpType.add
    BYPASS = mybir.AluOpType.bypass

    persist = ctx.enter_context(tc.tile_pool(name="persist", bufs=1))
    xT = persist.tile([P, 8, N], BF16, tag="xT")        # [c%128, c//128, n]
    wq = persist.tile([P, 8, 3 * DC], BF16, tag="wq")   # [c%128, c//128, col]
    wp = persist.tile([P, 4, C], BF16, tag="wp")        # [d%128, d//128, c]
    qT = persist.tile([P, 4, N], BF16, tag="qT")        # [d%128, pair, n]
    kT = persist.tile([P, 4, N], BF16, tag="kT")
    va = persist.tile([P, 16, DC], BF16, tag="va")      # [n%128, n//128, d]
    aT = persist.tile([P, 4, N], BF16, tag="aT")        # attn out^T

    # ---------------- consts, then DMAs, then PE warmup ----------------
    # consts first: they are tiny (~3us) but the warmup matmuls need
    # identb, and the iota/memsets run on the same engines that trigger
    # DMAs - behind 24 trigger ops they would delay the warmup ~15us.
    const = ctx.enter_context(tc.tile_pool(name="const", bufs=1))
    ident32 = const.tile([P, P], F32, tag="ident32")
    make_identity(nc, ident32)
    identb = const.tile([P, P], BF16, tag="identb")
    nc.vector.tensor_copy(identb[:], ident32[:])
    onesq = const.tile([P, 1], BF16, tag="onesq")
    nc.any.memset(onesq[:], 1.0)
    onescol = const.tile([P, 64], BF16, tag="onescol")
    nc.any.memset(onescol[:], 1.0)

    # wqkv first (the first qkv matmul needs ALL its row-chunks), then xT
    # slab by slab, then wproj; all spread over THREE trigger rings (sync/
    # scalar/gpsimd - the only DMA-capable engines, ~85 GB/s each) so the
    # per-ring drain rate doesn't serialize the input load.
    rings = [nc.sync, nc.scalar, nc.gpsimd]
    ring_i = [0]

    def dma_in(dst, src):
        rings[ring_i[0] % 3].dma_start(dst, src)
        ring_i[0] += 1

    def wq_cols(g):  # one 512-wide column group of wqkv for all row chunks
        for cc in range(8):
            dma_in(wq[:, cc, g * 512:(g + 1) * 512],
                   wqkv_d[cc * P:(cc + 1) * P, g * 512:(g + 1) * 512])

    def xt_half(h):     # half-rows: 2KB per partition line, efficient DMA
        for cc in range(8):
            dma_in(xT[:, cc, h * 1024:(h + 1) * 1024],
                   xT_d[cc * P:(cc + 1) * P, h * 1024:(h + 1) * 1024])

    wq_cols(1)          # k columns - the only weight phase 1 needs first
    xt_half(0)          # token slabs 0-1
    wq_cols(2)          # v columns
    xt_half(1)          # token slabs 2-3
    wq_cols(0)          # q columns
    for dc in range(4):
        dma_in(wp[:, dc, :], wproj_d[dc * P:(dc + 1) * P, :])

    with tc.tile_pool(name="ps_warm", bufs=1, space="PSUM") as ps_warm:
        warm = ps_warm.tile([P, P], F32, tag="warm")
        for _ in range(48):
            nc.tensor.matmul(warm[:], identb[:], identb[:])

    # ---------------- phase 1: qkv projection ----------------
    evac_flip = [0]

    def evac(dst, src):
        if evac_flip[0] % 2 == 0:
            nc.vector.tensor_copy(dst, src)
        else:
            nc.scalar.copy(dst, src)
        evac_flip[0] += 1

    with tc.tile_pool(name="ps1", bufs=6, space="PSUM") as ps1:
        def kq_pair(dst, colbase, dc0, ns):
            # two accumulation groups interleaved: each ldweights hides
            # under the other group's streaming matmul
            psA = ps1.tile([P, 512], F32, tag="ps1")
            psB = ps1.tile([P, 512], F32, tag="ps1")
            for cc in range(8):
                for j, ps in ((0, psA), (1, psB)):
                    col = colbase + (dc0 + j) * P
                    nc.tensor.matmul(
                        ps[:],
                        wq[:, cc, col:col + P],
                        xT[:, cc, ns * 512:(ns + 1) * 512],
                        start=(cc == 0), stop=(cc == 7),
                    )
            evac(dst[:, dc0, ns * 512:(ns + 1) * 512], psA[:])
            evac(dst[:, dc0 + 1, ns * 512:(ns + 1) * 512], psB[:])

        def v_pair(nck0):
            psA = ps1.tile([P, 512], F32, tag="ps1")
            psB = ps1.tile([P, 512], F32, tag="ps1")
            for cc in range(8):
                for j, ps in ((0, psA), (1, psB)):
                    nck = nck0 + j
                    nc.tensor.matmul(
                        ps[:],
                        xT[:, cc, nck * P:(nck + 1) * P],
                        wq[:, cc, 2 * DC:3 * DC],
                        start=(cc == 0), stop=(cc == 7),
                    )
            evac(va[:, nck0, :], psA[:])
            evac(va[:, nck0 + 1, :], psB[:])

        for ns in range(NSLABS):
            for dc0 in (0, 2):
                kq_pair(kT, DC, dc0, ns)
        for nck0 in range(0, 16, 2):
            v_pair(nck0)
        for ns in range(NSLABS):
            for dc0 in (0, 2):
                kq_pair(qT, 0, dc0, ns)

    # ---------------- phase 2: attention + proj, one flat pipeline ----------------
    with tc.tile_pool(name="st", bufs=3, space="PSUM") as st_pool, \
         tc.tile_pool(name="pv", bufs=1, space="PSUM") as pv_pool, \
         tc.tile_pool(name="dn", bufs=1, space="PSUM") as dn_pool, \
         tc.tile_pool(name="epool", bufs=18) as epool, \
         tc.tile_pool(name="nrm", bufs=2) as nrm_pool, \
         tc.tile_pool(name="oproj", bufs=2) as opool:

        queue = []        # exp'd chunks awaiting PV/dn: (s, p, ck, e)
        pair_acc = {}     # (s, p) -> (pv_tile, dn_tile), lazily allocated
        aux_sched = []    # (ready_G, thunk): chain ops spread one per boundary
        bc_sched = []     # (ready_G, s, p, aslc, rcb)
        proj_avail = []   # (s, nck, ct) proj tiles whose aT deps are emitted
        G = [0]           # global group counter

        def emit_bc(s, p, aslc, rcb, pool=None):
            # PE broadcast of the per-token reciprocal denominators (rows
            # 0/32 of rcb -> 64-row blocks), then in-place normalize of aT.
            if pool is None:
                bct = st_pool.tile([P, 2, 512], F32, tag="st",
                                   name=f"bc{s}_{p}")[:, 0, :]
            else:
                bct = pool.tile([P, 512], F32, tag="dn", name=f"bc{s}_{p}")[:]
            for sub in range(2):
                nc.tensor.matmul(
                    bct[64 * sub:64 * sub + 64, :],
                    onescol[32 * sub:32 * sub + 1, :],
                    rcb[32 * sub:32 * sub + 1, :],
                    tile_position=(32 * sub, 64 * sub),
                )
            nc.vector.scalar_tensor_tensor(
                aslc, aslc, 0.0, bct[:, :], BYPASS, MULT,
            )

        out_ring = [0]

        def emit_proj(s, nck, ct, pool):
            # one output-projection column tile, riding in a pv-/dn-bank
            # gap between that accumulator's last read and the next pair's
            # first write
            pp = pool.tile([P, 512], F32, tag=("pv" if pool is pv_pool else "dn"),
                           name=f"proj{s}_{nck}_{ct}")
            for dc in range(4):
                nc.tensor.matmul(
                    pp[:],
                    aT[:, dc, nck * P:(nck + 1) * P],
                    wp[:, dc, ct * 512:(ct + 1) * 512],
                    start=(dc == 0), stop=(dc == 3),
                )
            ot = opool.tile([P, 512], F32, tag="ot")
            evac(ot[:], pp[:])
            ring = nc.sync if out_ring[0] % 2 == 0 else nc.gpsimd
            out_ring[0] += 1
            ring.dma_start(
                out_d[nck * P:(nck + 1) * P, ct * 512:(ct + 1) * 512],
                ot[:],
            )

        def pop_proj(pool):
            if proj_avail:
                emit_proj(*proj_avail.pop(0), pool)

        def finish_pair(s, p, pv, dn):
            # The pair-end chain is spread one op per group boundary so it
            # never bunches up in front of the next chunks' exps in the
            # ACT/DVE FIFOs (each exp completion releases an st tile; a
            # bunched chain stalls S for ~1us).  pv/dn bank gaps between
            # their last read and the next pair's first write carry one
            # proj tile each.
            aslc = aT[:, p, s * 512:(s + 1) * 512]
            dsb = nrm_pool.tile([33, 512], F32, tag="dsb", name=f"dsb{s}_{p}")
            dadd = nrm_pool.tile([33, 512], F32, tag="dadd", name=f"dadd{s}_{p}")
            rc32 = nrm_pool.tile([33, 512], F32, tag="rc32", name=f"rc32_{s}_{p}")
            rcb = nrm_pool.tile([33, 512], BF16, tag="rcb", name=f"rcb{s}_{p}")

            def aux1():
                nc.scalar.copy(aslc, pv[:])
            def aux2():
                nc.scalar.copy(dsb[:], dn[64:97, :])
                nc.vector.scalar_tensor_tensor(
                    dadd[:], dn[0:33, :], 0.0, dsb[:], BYPASS, ADD,
                )
                pop_proj(pv_pool)
            def aux3():
                nc.vector.reciprocal_approx_fast(rc32[:], dadd[:])
                nc.vector.tensor_copy(rcb[:], rc32[:])
                pop_proj(dn_pool)
            aux_sched.append((G[0] + 1, aux1))
            aux_sched.append((G[0] + 2, aux2))
            aux_sched.append((G[0] + 3, aux3))
            bc_sched.append((G[0] + 5, s, p, aslc, rcb))

        def flush_batch():
            # pop up to 4 same-pair chunks; batching keeps runs of
            # same-weight-class matmuls long (each S<->PV<->dn class switch
            # exposes ~100ns of non-overlapped LDWEIGHTS)
            items = [queue.pop(0)]
            while queue and len(items) < 8 and queue[0][:2] == items[0][:2]:
                items.append(queue.pop(0))
            assert len(items) % 2 == 0
            s, p = items[0][:2]
            if (s, p) not in pair_acc:
                pair_acc[(s, p)] = (
                    pv_pool.tile([P, 512], F32, tag="pv", name=f"pv{s}_{p}"),
                    dn_pool.tile([P, 512], F32, tag="dn", name=f"dn{s}_{p}"),
                )
            pv, dn = pair_acc[(s, p)]
            for _, _, ck, e in items:
                for sub in range(2):
                    o = 64 * sub
                    h = 2 * p + sub
                    nc.tensor.matmul(
                        pv[o:o + 64, :],
                        va[:, ck, 64 * h:64 * h + 64],
                        e[:, sub, :],
                        start=(ck == 0), stop=(ck == 15),
                        tile_position=(0, o),
                    )
            for _, _, ck, e in items:
                ro = 64 * (ck % 2)
                for sub in range(2):
                    r = ro + 32 * sub
                    nc.tensor.matmul(
                        dn[r:r + 1, :],
                        onesq[:, :],
                        e[:, sub, :],
                        start=(ck < 2), stop=(ck >= 14),
                        tile_position=(0, r),
                    )
            if items[-1][2] == 15:
                finish_pair(s, p, pv, dn)

        def service():
            while aux_sched and aux_sched[0][0] <= G[0]:
                aux_sched.pop(0)[1]()
            if bc_sched and bc_sched[0][0] <= G[0]:
                _, s, p, aslc, rcb = bc_sched.pop(0)
                emit_bc(s, p, aslc, rcb)
                if p == 3 and s < 3:
                    for nck in range(4 * s, 4 * s + 4):
                        for ct in range(2):
                            proj_avail.append((s, nck, ct))

        for s in range(NSLABS):
            for p in range(4):
                for g in range(8):
                    service()
                    # flush BEFORE this group's S chunks: the flush's deps
                    # are long ready, so it executes during what would
                    # otherwise be the S chunks' st-tile wait
                    while len(queue) > DEPTH:
                        flush_batch()
                    for ck in (2 * g, 2 * g + 1):
                        # exp emitted immediately after its S chunk so the
                        # st tile is released as early as possible
                        st = st_pool.tile([P, 2, 512], F32, tag="st")
                        for sub in range(2):
                            o = 64 * sub
                            nc.tensor.matmul(
                                st[:, sub, :],
                                kT[o:o + 64, p, ck * P:(ck + 1) * P],
                                qT[o:o + 64, p, s * 512:(s + 1) * 512],
                                tile_position=(o, 0),
                            )
                        e = epool.tile([P, 2, 512], BF16, tag="e")
                        if ck in DVE_CKS or (ck == 13 and (4 * s + p) % 2 == 1):
                            nc.vector.tensor_scalar(
                                e.bitcast(I16)[:], st[:], SCH_A, SCH_B,
                                MULT, ADD,
                            )
                        else:
                            nc.scalar.activation(
                                e[:], st[:],
                                mybir.ActivationFunctionType.Exp, scale=0.125,
                            )
                        queue.append((s, p, ck, e))
                    G[0] += 1

        # ---- tail: drain the queue, remaining bc/stt, slab-3 proj ----
        while queue:
            flush_batch()
            G[0] += 1
            service()
        while aux_sched:
            aux_sched.pop(0)[1]()
        # all bc except the last pair's (its chain is still in flight on DVE)
        while len(bc_sched) > 1:
            _, s, p, aslc, rcb = bc_sched.pop(0)
            emit_bc(s, p, aslc, rcb, dn_pool)
        while proj_avail:
            emit_proj(*proj_avail.pop(0), dn_pool)
        # slab-3 proj, split accumulation: the dc=0..2 partials only need
        # pairs 0-2 normalized, so they keep the PE busy while the last
        # pair's denominator chain drains; dc=3 lands after the final stt.
        partials = []
        for nck in range(12, 15):
            pp = st_pool.tile([P, 2, 512], F32, tag="st", name=f"projt{nck}")
            for ct in range(2):
                for dc in range(3):
                    nc.tensor.matmul(
                        pp[:, ct, :],
                        aT[:, dc, nck * P:(nck + 1) * P],
                        wp[:, dc, ct * 512:(ct + 1) * 512],
                        start=(dc == 0), stop=False,
                    )
            partials.append((nck, pp))
        _, s, p, aslc, rcb = bc_sched.pop(0)
        emit_bc(s, p, aslc, rcb, dn_pool)
        for i, (nck, pp) in enumerate(partials):
            for ct in range(2):
                nc.tensor.matmul(
                    pp[:, ct, :],
                    aT[:, 3, nck * P:(nck + 1) * P],
                    wp[:, 3, ct * 512:(ct + 1) * 512],
                    start=False, stop=True,
                )
            ot = opool.tile([P, 2, 512], F32, tag="ott")
            evac(ot[:], pp[:])
            ring = nc.sync if i % 2 == 0 else nc.gpsimd
            ring.dma_start(out_d[nck * P:(nck + 1) * P, :], ot[:])
        pp = st_pool.tile([P, 2, 512], F32, tag="st", name="projt15")
        for ct in range(2):
            for dc in range(4):
                nc.tensor.matmul(
                    pp[:, ct, :],
                    aT[:, dc, 15 * P:16 * P],
                    wp[:, dc, ct * 512:(ct + 1) * 512],
                    start=(dc == 0), stop=(dc == 3),
                )
        ot = opool.tile([P, 2, 512], F32, tag="ott")
        evac(ot[:], pp[:])
        nc.gpsimd.dma_start(out_d[15 * P:16 * P, :], ot[:])


def shard_inputs(x, W_qkv, W_proj):
    """Full inputs -> 8 per-core in_maps. Core c: batch c//2, head-group c%2."""
    x = np.asarray(x, dtype=np.float32)
    W_qkv = np.asarray(W_qkv, dtype=np.float32)
    W_proj = np.asarray(W_proj, dtype=np.float32)
    bf = ml_dtypes.bfloat16
    in_maps = []
    for core in range(8):
        b, g = core // 2, core % 2
        cols = slice(g * DC, (g + 1) * DC)
        w = np.concatenate(
            [W_qkv[:, 0:C][:, cols], W_qkv[:, C:2 * C][:, cols],
             W_qkv[:, 2 * C:3 * C][:, cols]],
            axis=1,
        )
        in_maps.append({
            "xT": np.ascontiguousarray(x[b].T).astype(bf),
            "wqkv": np.ascontiguousarray(w).astype(bf),
            "wproj": np.ascontiguousarray(W_proj[g * DC:(g + 1) * DC, :]).astype(bf),
        })
    return in_maps


def unshard_output(results, b_proj):
    b_proj = np.asarray(b_proj, dtype=np.float32)
    out = np.empty((4, N, C), dtype=np.float32)
    for b in range(4):
        out[b] = results[2 * b]["out"] + results[2 * b + 1]["out"] + b_proj[None, :]
    return out


_NC_CACHE = []


def kernel(x, W_qkv, W_proj, b_proj, trace=False):
    in_maps = shard_inputs(x, W_qkv, W_proj)
    if not _NC_CACHE:
        _NC_CACHE.append(build_program())
    nc = _NC_CACHE[0]
    res = run_bass_kernel_spmd(nc, in_maps, core_ids=list(range(8)), trace=trace)
    out = unshard_output(res.results, b_proj)
    if trace:
        return out, res
    return out
